# revision 12
# baseline (speedup 1.0000x reference)
"""DTM decoder kernel for one TRN2 chip (8 NeuronCores), tensor-parallel
over the vocab dimension.

Math (reference):
    logits[t,k,v] = sum_e topic_emb[t,k,e] * word_emb[v,e]        (T*K=500, V=50000)
    betas = softmax(logits, axis=v)
    out[b,:] = theta[b,:] @ betas[time_index[b]]                  (B=256)

Parallelization: shard V across 8 cores (V_c = 6250). Each core, flash-style:
  1. matmul1 per (tk-tile, v-chunk): logits chunk in PSUM (f32 accum over E),
     DVE chunk row-max (negated), ScalarE evicts PSUM with exp(l - m_chunk)
     into the persistent P tiles and accumulates the chunk row-sum.
     The exp runs concurrently with the remaining matmuls.
  2. tiny per-tile combines give local stats (m_c, s_c); a 4KB AllGather
     shares them; each core recomputes global (m_g, s_g).
  3. matmul2 per v-chunk j: theta'_j[tk,b] = theta[tk,b]*exp(m_chunk_j - m_g)/s_g
     (per-chunk scale absorbs both the flash rescale and the softmax
     normalization), out_chunk = theta'_j^T @ P_j.

Host side: word_embeddings is passed per-core pre-transposed ([E, V_c]) so the
contraction dim lands on SBUF partitions with no on-device transpose;
time_index gather is folded into a (TK, B) theta matrix on host (tiny).
Matmuls run as float32r (fp32 storage, reduced-precision multiply, full PE
rate); set DTM_MM1/DTM_MM2=f32 for exact-but-4x-slower.
"""

import os
import sys

if "/opt/trn_rl_repo" not in sys.path:
    sys.path.insert(0, "/opt/trn_rl_repo")

import numpy as np

from concourse import bacc, mybir, tile
from concourse.bass_utils import run_bass_kernel_spmd

B, V, K, T, E = 256, 50000, 50, 10, 1024
TK = T * K  # 500
N_CORES = 8
VC = V // N_CORES  # 6250
P = 128

TK_CHUNKS = [(0, 128), (128, 128), (256, 128), (384, 116)]
E_CHUNKS = 8  # E / 128
# All chunks >= 256 (float32r full rate) and even (fp32r ISA restriction).
V_CHUNKS = [(i * 512, 512) for i in range(11)] + [(5632, 310), (5942, 308)]
assert sum(n for _, n in V_CHUNKS) == VC

F32 = mybir.dt.float32
Exp = mybir.ActivationFunctionType.Exp

_MM1_DT = {"f32": F32, "f32r": mybir.dt.float32r}[os.environ.get("DTM_MM1", "f32r")]
_MM2_DT = {"f32": F32, "f32r": mybir.dt.float32r}[os.environ.get("DTM_MM2", "f32r")]


def build(vc=VC, v_chunks=None, debug=False):
    if v_chunks is None:
        v_chunks = V_CHUNKS
    nvc = len(v_chunks)
    nc = bacc.Bacc("TRN2", target_bir_lowering=False, debug=debug, num_devices=N_CORES)

    wembT = nc.dram_tensor("wembT", [E, vc], _MM1_DT, kind="ExternalInput").ap()
    topicT = nc.dram_tensor("topicT", [E, TK], _MM1_DT, kind="ExternalInput").ap()
    thetaT = nc.dram_tensor("thetaT", [TK, B], F32, kind="ExternalInput").ap()
    out = nc.dram_tensor("out", [B, vc], F32, kind="ExternalOutput").ap()

    # stats layout: [0:512] row-max m_c, [512:1024] row-sum s_c (500 used)
    stats_local = nc.dram_tensor("stats_local", [1, 1024], F32)
    stats_all = nc.dram_tensor("stats_all", [N_CORES, 1024], F32, addr_space="Shared")

    with tile.TileContext(nc) as tc:
        with (
            tc.tile_pool(name="pbig", bufs=1) as pbig,
            tc.tile_pool(name="const", bufs=1) as const,
            tc.tile_pool(name="wpool", bufs=16) as wpool,
            tc.tile_pool(name="thp", bufs=16) as thp,
            tc.tile_pool(name="opool", bufs=4) as opool,
            tc.tile_pool(name="psp", bufs=4, space="PSUM") as psp,
        ):
            # preload the exp table set on ScalarE while the first DMAs run
            warm = const.tile([P, 2], F32, tag="warm", name="warm")
            nc.vector.memset(warm[:], 0.0)
            nc.scalar.activation(warm[:], warm[:], Exp)

            topic_sb = []
            for e in range(E_CHUNKS):
                tt = const.tile([P, TK], _MM1_DT, tag=f"topic{e}", name=f"topic{e}")
                nc.sync.dma_start(out=tt[:], in_=topicT[e * P : (e + 1) * P, :])
                topic_sb.append(tt)

            theta_sb, p_t, mrun, sloc, negmm, smat, msfull = [], [], [], [], [], [], []
            for i, (r0, rows) in enumerate(TK_CHUNKS):
                th = const.tile([P, B], F32, tag=f"theta{i}", name=f"theta{i}")
                theta_sb.append(th)
                p_t.append(pbig.tile([P, vc], _MM2_DT, tag=f"P{i}", name=f"P{i}"))
                ms = const.tile([P, 2], F32, tag=f"ms{i}", name=f"ms{i}")
                nc.vector.memset(ms[:, 0:1], 0.0)  # padded rows: m_c = 0
                nc.vector.memset(ms[:, 1:2], 1.0)  # padded rows: s_c = 1
                mrun.append(ms[:, 0:1])
                sloc.append(ms[:, 1:2])
                msfull.append(ms)
                nm = const.tile([P, nvc], F32, tag=f"negmm{i}", name=f"negmm{i}")
                negmm.append(nm)
                sm = const.tile([P, nvc], F32, tag=f"smat{i}", name=f"smat{i}")
                smat.append(sm)

            # --- phase 1: logits chunks; fused exp-evict (flash style) ---
            for vi, (v0, nv) in enumerate(v_chunks):
                w = []
                for e in range(E_CHUNKS):
                    wt = wpool.tile([P, 512], _MM1_DT, tag="w", name="w")
                    nc.sync.dma_start(
                        out=wt[:, :nv], in_=wembT[e * P : (e + 1) * P, v0 : v0 + nv]
                    )
                    w.append(wt)
                for i, (r0, rows) in enumerate(TK_CHUNKS):
                    ps = psp.tile([P, 512], F32, tag="ps1", name="ps1", bufs=4)
                    for e in range(E_CHUNKS):
                        nc.tensor.matmul(
                            ps[:rows, :nv],
                            lhsT=topic_sb[e][:, r0 : r0 + rows],
                            rhs=w[e][:, :nv],
                            start=(e == 0),
                            stop=(e == E_CHUNKS - 1),
                        )
                    # -chunk_rowmax (DVE), then exp-evict + chunk rowsum (ScalarE)
                    nc.vector.reduce_max(
                        negmm[i][:rows, vi : vi + 1],
                        ps[:rows, :nv],
                        axis=mybir.AxisListType.X,
                        negate=True,
                    )
                    nc.scalar.activation(
                        p_t[i][:rows, v0 : v0 + nv],
                        ps[:rows, :nv],
                        Exp,
                        bias=negmm[i][:rows, vi : vi + 1],
                        accum_out=smat[i][:rows, vi : vi + 1],
                    )

            # theta loads (needed only by phase 4; emitted late so the
            # startup DMA bandwidth goes to topic + first wemb slabs)
            for i, (r0, rows) in enumerate(TK_CHUNKS):
                nc.sync.dma_start(
                    out=theta_sb[i][:rows, :], in_=thetaT[r0 : r0 + rows, :]
                )

            # --- phase 2: local stats + allgather ---
            for i, (r0, rows) in enumerate(TK_CHUNKS):
                # m_c = max_j m_j = -(min_j negm_j)
                nc.vector.tensor_reduce(
                    out=mrun[i][:rows],
                    in_=negmm[i][:rows, :nvc],
                    op=mybir.AluOpType.min,
                    axis=mybir.AxisListType.X,
                    negate=True,
                )
                nmc = const.tile([P, 1], F32, tag=f"nmc{i}", name=f"nmc{i}")
                nc.vector.tensor_scalar_mul(nmc[:rows, :], mrun[i][:rows], -1.0)
                # s_c = sum_j s_j * exp(m_j - m_c);  m_j = -negmm[:, j]
                wj = const.tile([P, nvc], F32, tag=f"wj{i}", name=f"wj{i}")
                nc.scalar.activation(
                    wj[:rows, :nvc],
                    negmm[i][:rows, :nvc],
                    Exp,
                    bias=nmc[:rows, :],
                    scale=-1.0,
                )
                nc.vector.tensor_mul(
                    wj[:rows, :nvc], wj[:rows, :nvc], smat[i][:rows, :nvc]
                )
                nc.vector.reduce_sum(
                    sloc[i][:rows], wj[:rows, :nvc], axis=mybir.AxisListType.X
                )
                # one packed DMA per tile: [m_p, s_p] interleaved in DRAM
                nc.gpsimd.dma_start(
                    out=stats_local[0, i * 2 * P : (i + 1) * 2 * P],
                    in_=msfull[i][:, :],
                )
            nc.gpsimd.collective_compute(
                "AllGather",
                mybir.AluOpType.bypass,
                replica_groups=[list(range(N_CORES))],
                ins=[stats_local[:].opt()],
                outs=[stats_all[:].opt()],
            )

            # --- phase 3: global stats; per-chunk scale matrix G ---
            # stats_all[c, i*256 + 2p + j]: m (j=0) / s (j=1) for tile i, row p
            stats_t = stats_all[:].rearrange("c (v two) -> v two c", two=2)  # [512,2,8]
            gmat = []
            for i, (r0, rows) in enumerate(TK_CHUNKS):
                mt = const.tile([P, N_CORES], F32, tag=f"mt{i}", name=f"mt{i}")
                st = const.tile([P, N_CORES], F32, tag=f"st{i}", name=f"st{i}")
                nc.sync.dma_start(out=mt[:], in_=stats_t[i * P : (i + 1) * P, 0, :])
                nc.sync.dma_start(
                    out=st[:], in_=stats_t[i * P : (i + 1) * P, 1, :]
                )
                nmg = const.tile([P, 1], F32, tag=f"nmg{i}", name=f"nmg{i}")
                nc.vector.reduce_max(
                    nmg[:], mt[:], axis=mybir.AxisListType.X, negate=True
                )
                wt = const.tile([P, N_CORES], F32, tag=f"wt{i}", name=f"wt{i}")
                nc.scalar.activation(wt[:], mt[:], Exp, bias=nmg[:])
                nc.vector.tensor_mul(wt[:], wt[:], st[:])
                sg = const.tile([P, 1], F32, tag=f"sg{i}", name=f"sg{i}")
                nc.vector.reduce_sum(sg[:], wt[:], axis=mybir.AxisListType.X)
                rg = const.tile([P, 1], F32, tag=f"rg{i}", name=f"rg{i}")
                nc.vector.reciprocal(rg[:], sg[:])
                # G[:, j] = exp(m_j - m_g) / s_g  (m_j = -negmm[:, j])
                g = const.tile([P, nvc], F32, tag=f"g{i}", name=f"g{i}")
                nc.scalar.activation(
                    g[:rows, :nvc],
                    negmm[i][:rows, :nvc],
                    Exp,
                    bias=nmg[:rows, :],
                    scale=-1.0,
                )
                nc.vector.tensor_scalar_mul(
                    g[:rows, :nvc], g[:rows, :nvc], rg[:rows, :]
                )
                gmat.append(g)

            # --- phase 4: out[b, v_j] = sum_tk theta[tk,b]*G[tk,j] * P[tk,v_j] ---
            for vi, (v0, nv) in enumerate(v_chunks):
                thv = []
                for i, (r0, rows) in enumerate(TK_CHUNKS):
                    tv = thp.tile([P, B], _MM2_DT, tag="thv", name="thv")
                    nc.vector.tensor_scalar_mul(
                        tv[:rows, :], theta_sb[i][:rows, :], gmat[i][:rows, vi : vi + 1]
                    )
                    thv.append(tv)
                for b0 in range(0, B, P):
                    ps = psp.tile([P, 512], F32, tag="ps2", name="ps2", bufs=4)
                    for i, (r0, rows) in enumerate(TK_CHUNKS):
                        nc.tensor.matmul(
                            ps[:, :nv],
                            lhsT=thv[i][:rows, b0 : b0 + P],
                            rhs=p_t[i][:rows, v0 : v0 + nv],
                            start=(i == 0),
                            stop=(i == 3),
                        )
                    ot = opool.tile([P, 512], F32, tag="ot", name="ot")
                    nc.scalar.copy(ot[:, :nv], ps[:, :nv])
                    nc.sync.dma_start(
                        out=out[b0 : b0 + P, v0 : v0 + nv], in_=ot[:, :nv]
                    )

    nc.compile()
    return nc


_NC_CACHE = None


def _get_nc():
    global _NC_CACHE
    if _NC_CACHE is None:
        _NC_CACHE = build()
    return _NC_CACHE


def kernel(theta, word_embeddings, topic_embeddings, time_index):
    theta = np.ascontiguousarray(np.asarray(theta), dtype=np.float32)
    wemb = np.asarray(word_embeddings, dtype=np.float32)
    topic = np.asarray(topic_embeddings, dtype=np.float32)
    ti = np.asarray(time_index).astype(np.int64)

    # time-gathered theta, transposed: thetaT[t*K + k, b] = theta[b, k] iff ti[b] == t
    thetaT = np.zeros((TK, B), dtype=np.float32)
    rows = (ti[:, None] * K + np.arange(K)[None, :]).ravel()
    cols = np.repeat(np.arange(B), K)
    thetaT[rows, cols] = theta.ravel()

    topicT = np.ascontiguousarray(topic.reshape(TK, E).T)  # [E, TK]

    in_maps = []
    for c in range(N_CORES):
        shard = np.ascontiguousarray(wemb[c * VC : (c + 1) * VC, :].T)  # [E, VC]
        in_maps.append({"wembT": shard, "topicT": topicT, "thetaT": thetaT})

    nc = _get_nc()
    res = run_bass_kernel_spmd(nc, in_maps, core_ids=list(range(N_CORES)))
    return np.concatenate([res.results[c]["out"] for c in range(N_CORES)], axis=1)
